# revision 19
# baseline (speedup 1.0000x reference)
"""BD3LM block-diffusion decoder layer on 8 trn2 NeuronCores — v2 (bf16).

Sharding: core = 2*b + g  (b = batch 0..3, g = head-group 0..1, 8 heads each).
All matmuls bf16 (inputs converted on host; hd^-0.5 folded into Wq; bk dropped
— a per-query constant score shift is softmax-invariant). f32 PSUM accum.

Key structures vs v1:
  - Block masks applied ADDITIVELY on the PE: scores += A^T B where A[r,k] =
    [k//4 == r] (rank-32 indicator) and B[r,q] carries NEG on disallowed
    (block_r, q) pairs. One extra 128-col matmul per boundary tile; exp then
    covers a whole (head, half, j) span in ONE activation instruction.
  - Per-head-pair QKV projection, emission-interleaved with the previous
    pair's attention so the PE never waits on ACT's exp stream.
  - Softmax denominators via ones-columns in v (ctx row 64); reciprocal rows
    are broadcast across 64 partitions by a DRAM round-trip DMA; normalize is
    fused into the ctx eviction multiply on DVE.
  - Phase C (O-projection) interleaved into the last head's attention tail.
"""

import numpy as np
import ml_dtypes

import concourse.bass as bass
import concourse.mybir as mybir
import concourse.tile as tile
from concourse import bacc
from concourse.bass_utils import run_bass_kernel_spmd

F32 = mybir.dt.float32
BF16 = mybir.dt.bfloat16
Act = mybir.ActivationFunctionType

B, T, D = 4, 2048, 1024
H, HD = 16, 64
L = T // 2           # 1024
BS = 4               # block size
P = 128
NT = L // P          # 8 key/query tiles per half
PAIRS = 4            # head-pairs per core
KC = D // P          # 8 contraction chunks
NEG = -60.0          # additive mask value

REPEAT = 1  # loop whole computation inside the NEFF (timing experiments only)
DBG = False
INTERLEAVE = True

_CACHE = {}


def _chunks512(a0, a1):
    """Split [a0, a1) at multiples of 512 (PSUM bank boundaries)."""
    res = []
    while a0 < a1:
        b1 = min(a1, (a0 // 512 + 1) * 512)
        res.append((a0, b1))
        a0 = b1
    return res


def _mask_arrays():
    """A [32,128] indicator; B patterns [32,128] (strict/incl); Bd8 [32,1024]."""
    A = np.zeros((32, P), np.float32)
    A[np.arange(P) // BS, np.arange(P)] = 1.0
    r = np.arange(32)[:, None]
    qb = (np.arange(P) // BS)[None, :]
    Bs = np.where(r >= qb, NEG, 0.0).astype(np.float32)   # xt q vs x0 k: allow r < qb
    Bi = np.where(r > qb, NEG, 0.0).astype(np.float32)    # x0 q vs x0 k: allow r <= qb
    Bd = np.where(r != qb, NEG, 0.0).astype(np.float32)   # xt q vs xt k: allow r == qb
    Bd8 = np.tile(Bd, (1, NT))                             # [32, 1024]
    bf = ml_dtypes.bfloat16
    return A.astype(bf), Bs.astype(bf), Bi.astype(bf), Bd8.astype(bf)


def _build():
    import concourse.tile_utils as tile_utils

    tile_utils.max_sbuf_usage = 204 * 1024

    nc = bacc.Bacc("TRN2", target_bir_lowering=False, debug=False, num_devices=8)

    xT = nc.dram_tensor("xT", [D, T], BF16, kind="ExternalInput").ap()
    wq = nc.dram_tensor("wq", [D, D // 2], BF16, kind="ExternalInput").ap()
    wk = nc.dram_tensor("wk", [D, D // 2], BF16, kind="ExternalInput").ap()
    wv = nc.dram_tensor("wv", [D, D // 2], BF16, kind="ExternalInput").ap()
    wo = nc.dram_tensor("wo", [D // 2, D], BF16, kind="ExternalInput").ap()
    bqs = nc.dram_tensor("bqs", [D // 2], F32, kind="ExternalInput").ap()
    out = nc.dram_tensor("out", [T, D], F32, kind="ExternalOutput").ap()
    scratch = nc.dram_tensor(
        "scratch", [16, L], F32, kind="ExternalOutput" if DBG else "Internal"
    ).ap()
    dbg = {}
    if DBG:
        for nm, shp, dt in (
            ("dbg_qT", [P, PAIRS, T], F32),
            ("dbg_kT", [P, PAIRS, T], F32),
            ("dbg_v", [P, PAIRS, T // P, 2 * (HD + 1)], F32),
            ("dbg_ctxT", [P, PAIRS, T], F32),
        ):
            dbg[nm] = nc.dram_tensor(nm, shp, dt, kind="ExternalOutput").ap()

    mA_np, mBs_np, mBi_np, mBd8_np = _mask_arrays()
    mA_d = nc.inline_tensor(mA_np, "mA_d").ap()
    mBs_d = nc.inline_tensor(mBs_np, "mBs_d").ap()
    mBi_d = nc.inline_tensor(mBi_np, "mBi_d").ap()
    mBd_d = nc.inline_tensor(mBd8_np, "mBd_d").ap()

    views = dict(
        xT_v=xT.rearrange("(kc p) t -> p kc t", p=P),      # [128, 8, 2048]
        wq_v=wq.rearrange("(kc p) m -> p kc m", p=P),      # [128, 8, 512]
        wk_v=wk.rearrange("(kc p) m -> p kc m", p=P),
        wv_v=wv.rearrange("(kc p) m -> p kc m", p=P),
        wo_v=wo.rearrange("(cc p) n -> p cc n", p=P),      # [128, 4, 1024]
        bqs_v=bqs.rearrange("(c p) -> p c", p=P),          # [128, 4]
        mA=mA_d, mBs=mBs_d, mBi=mBi_d, mBd=mBd_d,
        out=out, scratch=scratch, dbg=dbg,
    )

    with tile.TileContext(nc) as tc:
        with tc.tile_pool(name="persist", bufs=1) as pers:
            st = dict(
                x_sb=pers.tile([P, KC, T], BF16, name="x_sb"),
                qT=pers.tile([P, PAIRS, T], BF16, name="qT"),
                kT=pers.tile([P, PAIRS, T], BF16, name="kT"),
                v_sb=pers.tile([P, PAIRS, T // P, 2 * (HD + 1)], BF16, name="v_sb"),
                ctxT=pers.tile([P, PAIRS, T], BF16, name="ctxT"),
                wq_sb=pers.tile([P, KC, D // 2], BF16, name="wq_sb"),
                wk_sb=pers.tile([P, KC, D // 2], BF16, name="wk_sb"),
                wv_sb=pers.tile([P, KC, D // 2], BF16, name="wv_sb"),
                wo_sb=pers.tile([P, PAIRS, D], BF16, name="wo_sb"),
                bq_sb=pers.tile([P, PAIRS], F32, name="bq_sb"),
                mA_sb=pers.tile([32, P], BF16, name="mA_sb"),
                mBs_sb=pers.tile([32, P], BF16, name="mBs_sb"),
                mBi_sb=pers.tile([32, P], BF16, name="mBi_sb"),
                mBd_sb=pers.tile([32, NT * P], BF16, name="mBd_sb"),
            )
            # ones columns (64 and 129) for the softmax denominators
            vv = st["v_sb"]
            nc.vector.memset(vv[:, :, :, HD : HD + 1], 1.0)
            nc.vector.memset(vv[:, :, :, 2 * HD + 1 : 2 * HD + 2], 1.0)
            st["ones_t"] = pers.tile([1, HD], F32, name="ones_t")
            nc.vector.memset(st["ones_t"], 1.0)
            nc.sync.dma_start(st["mA_sb"], views["mA"])
            nc.sync.dma_start(st["mBs_sb"], views["mBs"])
            nc.sync.dma_start(st["mBi_sb"], views["mBi"])
            nc.sync.dma_start(st["mBd_sb"], views["mBd"])
            nc.sync.dma_start(st["bq_sb"], views["bqs_v"])

            for _rep in range(REPEAT):
                _phases(nc, tc, st, views)

    nc.compile()
    return nc


def _phases(nc, tc, st, views):
    x_sb, qT, kT, v_sb, ctxT = (
        st["x_sb"], st["qT"], st["kT"], st["v_sb"], st["ctxT"]
    )
    scratch, out = views["scratch"], views["out"]

    # input DMAs, ordered by first use: pair-0 weights + x slab 0 first
    cols0 = slice(0, P)
    nc.sync.dma_start(st["wq_sb"][:, :, cols0], views["wq_v"][:, :, cols0])
    nc.sync.dma_start(st["wk_sb"][:, :, cols0], views["wk_v"][:, :, cols0])
    nc.sync.dma_start(st["wv_sb"][:, :, cols0], views["wv_v"][:, :, cols0])
    for s4 in range(4):  # slab 0 split fine so the first chunk lands early
        nc.sync.dma_start(
            x_sb[:, :, 128 * s4 : 128 * (s4 + 1)],
            views["xT_v"][:, :, 128 * s4 : 128 * (s4 + 1)],
        )
    for s in range(1, 4):
        nc.sync.dma_start(
            x_sb[:, :, 512 * s : 512 * (s + 1)],
            views["xT_v"][:, :, 512 * s : 512 * (s + 1)],
        )
    for p in range(1, PAIRS):
        cols = slice(P * p, P * (p + 1))
        nc.sync.dma_start(st["wq_sb"][:, :, cols], views["wq_v"][:, :, cols])
        nc.sync.dma_start(st["wk_sb"][:, :, cols], views["wk_v"][:, :, cols])
        nc.sync.dma_start(st["wv_sb"][:, :, cols], views["wv_v"][:, :, cols])
    nc.sync.dma_start(st["wo_sb"], views["wo_v"])

    from contextlib import ExitStack

    es = ExitStack()
    atp = es.enter_context(tc.tile_pool(name="atp", bufs=10))
    ctxs_p = es.enter_context(tc.tile_pool(name="ctxs_p", bufs=3))
    rbp = es.enter_context(tc.tile_pool(name="rbp", bufs=3))
    rcp = es.enter_context(tc.tile_pool(name="rcp", bufs=2))
    osbp = es.enter_context(tc.tile_pool(name="osbp", bufs=4))
    spsum = es.enter_context(tc.tile_pool(name="spsum", bufs=2, space="PSUM"))
    cpsum = es.enter_context(tc.tile_pool(name="cpsum", bufs=1, space="PSUM"))

    uid = [0]

    def nid():
        uid[0] += 1
        return uid[0]

    # ---------------- emission units ----------------

    def proj_units(apsum, p):
        """QKV projection for head-pair p as a list of small closures."""
        units = []
        cols = slice(P * p, P * (p + 1))

        def qk_chunk(dst, w_sb, tchunk, is_q):
            def go():
                ps = apsum.tile([P, 512], F32, tag="pp", name=f"pp{nid()}")
                for kc in range(KC):
                    nc.tensor.matmul(
                        ps,
                        w_sb[:, kc, cols],
                        x_sb[:, kc, 512 * tchunk : 512 * (tchunk + 1)],
                        start=(kc == 0),
                        stop=(kc == KC - 1),
                    )
                dv = dst[:, p, 512 * tchunk : 512 * (tchunk + 1)]
                if is_q:
                    nc.vector.tensor_scalar_add(dv, ps, st["bq_sb"][:, p : p + 1])
                else:
                    nc.vector.tensor_copy(dv, ps)
            return go

        def v_tile(tt):
            def go():
                ps = apsum.tile([P, P], F32, tag="pp", name=f"ppv{nid()}")
                xt = x_sb[:, :, P * tt : P * (tt + 1)]
                for kc in range(KC):
                    nc.tensor.matmul(
                        ps,
                        xt[:, kc, :],
                        st["wv_sb"][:, kc, cols],
                        start=(kc == 0),
                        stop=(kc == KC - 1),
                    )
                dv = v_sb[:, p, tt].rearrange("p (h c) -> p h c", c=HD + 1)[:, :, :HD]
                nc.vector.tensor_copy(dv, ps.rearrange("p (h c) -> p h c", c=HD))
            return go

        for tchunk in range(4):
            units.append(qk_chunk(qT, st["wq_sb"], tchunk, True))
            units.append(qk_chunk(kT, st["wk_sb"], tchunk, False))
            for t2 in range(4):
                units.append(v_tile(4 * tchunk + t2))
        return units

    def attn_head(h, ticks, fast_norm=False):
        """Emit attention for local head h; ticks[half]() at interleave slots."""
        p, r0 = h // 2, HD * (h % 2)
        qh = qT[r0 : r0 + HD, p, :]
        kh = kT[r0 : r0 + HD, p, :]
        vcol = (HD + 1) * (h % 2)

        for half in range(2):
            tick = ticks[half]
            ats = []
            # scores for all j (keys = x0 tiles)
            for j in range(NT):
                span = L - P * j
                kv = kh[:, L + P * j : L + P * (j + 1)]
                sc = spsum.tile([P, 1024], F32, tag="sc", name=f"sc{nid()}")
                q0 = L * half + P * j
                for c0 in range(0, span, 512):
                    c1 = min(span, c0 + 512)
                    nc.tensor.matmul(
                        sc[:, c0:c1], kv, qh[:, q0 + c0 : q0 + c1],
                        start=True, stop=True,
                    )
                mB = st["mBs_sb"] if half == 0 else st["mBi_sb"]
                nc.tensor.matmul(sc[:, 0:P], st["mA_sb"], mB, start=False, stop=True)
                at = atp.tile([P, 1024], BF16, tag="at", name=f"at{nid()}")[:, :span]
                nc.scalar.activation(at, sc[:, :span], Act.Exp)
                ats.append(at)
                if j % 2 == 1:
                    tick()
            # xt-xt block-diagonal scores (half 0 only)
            if half == 0:
                scd = spsum.tile([P, 1024], F32, tag="sc", name=f"scd{nid()}")
                for i in range(NT):
                    # start=True clears has_written for the WHOLE bank: only
                    # the first write per 512-col bank may set it
                    nc.tensor.matmul(
                        scd[:, P * i : P * (i + 1)],
                        kh[:, P * i : P * (i + 1)],
                        qh[:, P * i : P * (i + 1)],
                        start=(i % 4 == 0), stop=True,
                    )
                for c in range(2):
                    nc.tensor.matmul(
                        scd[:, 512 * c : 512 * (c + 1)],
                        st["mA_sb"],
                        st["mBd_sb"][:, 512 * c : 512 * (c + 1)],
                        start=False, stop=True,
                    )
                atd = atp.tile([P, 1024], BF16, tag="at", name=f"atd{nid()}")
                nc.scalar.activation(atd, scd, Act.Exp)
                tick()

            # ctx accumulation over j (+ diag for half 0); chunks at absolute
            # 512-boundaries of the ctx tile (PSUM banks)
            ctx = cpsum.tile([HD + 1, L], F32, tag="ctx", name=f"ctx{nid()}")
            for j in range(NT):
                vj = v_sb[:, p, NT + j, vcol : vcol + HD + 1]
                for a0, a1 in _chunks512(P * j, L):
                    c0 = a0 - P * j
                    last = half == 1 and (
                        (a1 <= 512 and j == 3) or (a0 >= 512 and j == NT - 1)
                    )
                    nc.tensor.matmul(
                        ctx[:, a0:a1], vj, ats[j][:, c0 : c0 + (a1 - a0)],
                        start=(j == 0), stop=last,
                    )
                if j % 2 == 1:
                    tick()
            if half == 0:
                for i in range(NT):
                    vi = v_sb[:, p, i, vcol : vcol + HD + 1]
                    nc.tensor.matmul(
                        ctx[:, P * i : P * (i + 1)], vi, atd[:, P * i : P * (i + 1)],
                        start=False, stop=(i == 3 or i == NT - 1),
                    )
                tick()

            # evict ctx, 1/denom, DRAM-trip partition broadcast, normalize.
            # Split the eviction per bank so ctx's PSUM frees earlier (WAR).
            ctx_s = ctxs_p.tile([HD + 1, L], F32, tag="cs", name=f"cs{nid()}")
            nc.vector.tensor_copy(ctx_s[:, 0:512], ctx[:, 0:512])
            nc.vector.tensor_copy(ctx_s[:, 512:L], ctx[:, 512:L])
            rc = rcp.tile([1, L], F32, tag="rc", name=f"rc{nid()}")
            with nc.allow_low_precision(reason="deliberate f32r recip"):
                nc.vector.reciprocal(rc, ctx_s[HD : HD + 1, :])
            dst = ctxT[r0 : r0 + HD, p, L * half : L * (half + 1)]
            if fast_norm:
                # tail fast path: PE broadcast of 1/denom (no DRAM trip)
                bc = spsum.tile([P, 1024], F32, tag="sc", name=f"bc{nid()}")
                for c0 in (0, 512):
                    nc.tensor.matmul(
                        bc[:HD, c0 : c0 + 512],
                        st["ones_t"].bitcast(mybir.dt.float32r),
                        rc[0:1, c0 : c0 + 512].bitcast(mybir.dt.float32r),
                        start=True, stop=True,
                    )
                nc.vector.tensor_mul(dst, ctx_s[:HD, :], bc[:HD, :])
            else:
                row = 2 * h + half
                nc.sync.dma_start(scratch[row : row + 1, :], rc)
                rb = rbp.tile([HD, L], F32, tag="rb", name=f"rb{nid()}")
                nc.sync.dma_start(rb, scratch[row : row + 1, :].to_broadcast((HD, L)))
                nc.vector.tensor_mul(dst, ctx_s[:HD, :], rb)
            tick()

    def oproj_units(opsum, half, dve_evict):
        """O-projection units for output rows of one half."""
        units = []

        def o_unit(tt, nk):
            def go():
                ops = opsum.tile([P, 512], F32, tag="op", name=f"op{nid()}")
                for cc in range(PAIRS):
                    nc.tensor.matmul(
                        ops,
                        ctxT[:, cc, P * tt : P * (tt + 1)],
                        st["wo_sb"][:, cc, 512 * nk : 512 * (nk + 1)],
                        start=(cc == 0),
                        stop=(cc == PAIRS - 1),
                    )
                osb = osbp.tile([P, 512], F32, tag="osb", name=f"osb{nid()}")
                if dve_evict:
                    nc.vector.tensor_copy(osb, ops)
                else:
                    nc.scalar.activation(osb, ops, Act.Copy)
                nc.sync.dma_start(
                    out[P * tt : P * (tt + 1), 512 * nk : 512 * (nk + 1)], osb
                )
            return go

        for tt in range(NT * half, NT * (half + 1)):
            for nk in range(2):
                units.append(o_unit(tt, nk))
        return units

    # ---------------- schedule ----------------

    def make_tick(pending):
        state = [0]

        def tick():
            state[0] += 1
            if pending and state[0] % 2 == 0:
                pending.pop(0)()
        return tick

    noop = lambda: None

    if INTERLEAVE:
        with tc.tile_pool(name="apsum", bufs=2, space="PSUM") as apsum:
            # warmup: keep the PE busy (and its clock hot) while the first
            # x-slab DMA lands; mask tiles arrive within ~1us
            for w in range(12):
                wps = apsum.tile([P, 512], F32, tag="pp", name=f"warm{nid()}")
                nc.tensor.matmul(
                    wps, st["mA_sb"], st["mBd_sb"][:, 0:512],
                    start=True, stop=True,
                )
            for u in proj_units(apsum, 0):
                u()
            for p in range(3):
                pending = proj_units(apsum, p + 1)
                tick = make_tick(pending)
                attn_head(2 * p, (tick, tick))
                attn_head(2 * p + 1, (tick, tick))
                for u in pending:
                    u()

        with tc.tile_pool(name="opsum", bufs=2, space="PSUM") as opsum:
            attn_head(6, (noop, noop))
            pending = oproj_units(opsum, 0, dve_evict=True)  # xt rows
            tick = make_tick(pending)
            attn_head(7, (noop, tick), fast_norm=True)
            for u in pending:
                u()
            for u in oproj_units(opsum, 1, dve_evict=False):  # x0 rows
                u()
    else:
        with tc.tile_pool(name="apsum", bufs=2, space="PSUM") as apsum:
            for p in range(4):
                for u in proj_units(apsum, p):
                    u()
        for h in range(8):
            attn_head(h, (noop, noop))
        with tc.tile_pool(name="opsum", bufs=2, space="PSUM") as opsum:
            for half in range(2):
                for u in oproj_units(opsum, half, dve_evict=False):
                    u()

    if views["dbg"]:
        dbg = views["dbg"]
        with tc.tile_pool(name="dbgp", bufs=2) as dbgp:
            for nm, src in (
                ("dbg_qT", qT), ("dbg_kT", kT), ("dbg_ctxT", ctxT)
            ):
                for pp in range(PAIRS):
                    t32 = dbgp.tile([P, T], F32, tag="d32", name=f"d{nid()}")
                    nc.vector.tensor_copy(t32, src[:, pp, :])
                    nc.sync.dma_start(dbg[nm][:, pp, :], t32)
            for pp in range(PAIRS):
                t32 = dbgp.tile([P, T // P * 2 * (HD + 1)], F32, tag="d32", name=f"dv{nid()}")
                tv = t32.rearrange("p (t c) -> p t c", c=2 * (HD + 1))
                nc.vector.tensor_copy(tv, v_sb[:, pp])
                nc.sync.dma_start(dbg["dbg_v"][:, pp], tv)

    es.close()


def kernel(x, Wq, bq, Wk, bk, Wv, bv, Wo, bo, block_size=4, **_):
    assert int(block_size) == BS
    bf = ml_dtypes.bfloat16
    x = np.asarray(x, np.float32)
    Wq, bq = np.asarray(Wq, np.float32), np.asarray(bq, np.float32)
    Wk = np.asarray(Wk, np.float32)
    Wv, bv = np.asarray(Wv, np.float32), np.asarray(bv, np.float32)
    Wo, bo = np.asarray(Wo, np.float32), np.asarray(bo, np.float32)

    if "nc" not in _CACHE:
        _CACHE["nc"] = _build()
    nc = _CACHE["nc"]

    scale = np.float32(HD ** -0.5)
    in_maps = []
    for core in range(8):
        b, g = core // 2, core % 2
        cols = slice(D // 2 * g, D // 2 * (g + 1))
        in_maps.append(
            {
                "xT": np.ascontiguousarray(x[b].T).astype(bf),
                "wq": np.ascontiguousarray(Wq[:, cols] * scale).astype(bf),
                "wk": np.ascontiguousarray(Wk[:, cols]).astype(bf),
                "wv": np.ascontiguousarray(Wv[:, cols]).astype(bf),
                "wo": np.ascontiguousarray(Wo[cols, :]).astype(bf),
                "bqs": np.ascontiguousarray(bq[cols]) * scale,
            }
        )

    _CACHE["last_in_maps"] = in_maps
    last_err = None
    for _attempt in range(6):
        try:
            res = run_bass_kernel_spmd(nc, in_maps, core_ids=list(range(8)), trace=False)
            break
        except Exception as e:  # transient NRT device flakes
            last_err = e
            msg = str(e)
            if "UNRECOVERABLE" not in msg and "UNAVAILABLE" not in msg:
                raise
            import time as _time

            import jax as _jax

            _time.sleep(5 * (_attempt + 1))
            try:
                _jax.clear_backends()
            except Exception:
                pass
    else:
        raise last_err

    corr = (bv @ Wo + bo).astype(np.float32)  # softmax rows sum to 1
    outv = np.empty((B, T, D), np.float32)
    for b in range(B):
        outv[b] = res.results[2 * b]["out"] + res.results[2 * b + 1]["out"] + corr
    return outv


if __name__ == "__main__":
    rng = np.random.default_rng(0)
    inputs = {
        "x": rng.standard_normal((B, T, D)).astype(np.float32),
        "Wq": (rng.standard_normal((D, D)) / 32).astype(np.float32),
        "bq": np.zeros(D, np.float32),
        "Wk": (rng.standard_normal((D, D)) / 32).astype(np.float32),
        "bk": np.zeros(D, np.float32),
        "Wv": (rng.standard_normal((D, D)) / 32).astype(np.float32),
        "bv": np.zeros(D, np.float32),
        "Wo": (rng.standard_normal((D, D)) / 32).astype(np.float32),
        "bo": np.zeros(D, np.float32),
    }
    o = kernel(**inputs)
    print("ran", o.shape, o.dtype, float(np.abs(o).max()))


# revision 24
# speedup vs baseline: 1.0675x; 1.0675x over previous
"""BD3LM block-diffusion decoder layer on 8 trn2 NeuronCores — v2 (bf16).

Sharding: core = 2*b + g  (b = batch 0..3, g = head-group 0..1, 8 heads each).
All matmuls bf16 (inputs converted on host; hd^-0.5 folded into Wq; bk dropped
— a per-query constant score shift is softmax-invariant). f32 PSUM accum.

Key structures vs v1:
  - Block masks applied ADDITIVELY on the PE: scores += A^T B where A[r,k] =
    [k//4 == r] (rank-32 indicator) and B[r,q] carries NEG on disallowed
    (block_r, q) pairs. One extra 128-col matmul per boundary tile; exp then
    covers a whole (head, half, j) span in ONE activation instruction.
  - Per-head-pair QKV projection, emission-interleaved with the previous
    pair's attention so the PE never waits on ACT's exp stream.
  - Softmax denominators via ones-columns in v (ctx row 64); reciprocal rows
    are broadcast across 64 partitions by a DRAM round-trip DMA; normalize is
    fused into the ctx eviction multiply on DVE.
  - Phase C (O-projection) interleaved into the last head's attention tail.
"""

import numpy as np
import ml_dtypes

import concourse.bass as bass
import concourse.mybir as mybir
import concourse.tile as tile
from concourse import bacc
from concourse.bass_utils import run_bass_kernel_spmd

F32 = mybir.dt.float32
BF16 = mybir.dt.bfloat16
Act = mybir.ActivationFunctionType

B, T, D = 4, 2048, 1024
H, HD = 16, 64
L = T // 2           # 1024
BS = 4               # block size
P = 128
NT = L // P          # 8 key/query tiles per half
PAIRS = 4            # head-pairs per core
KC = D // P          # 8 contraction chunks
NEG = -60.0          # additive mask value

REPEAT = 1  # loop whole computation inside the NEFF (timing experiments only)
DBG = False
INTERLEAVE = True

_CACHE = {}


def _chunks512(a0, a1):
    """Split [a0, a1) at multiples of 512 (PSUM bank boundaries)."""
    res = []
    while a0 < a1:
        b1 = min(a1, (a0 // 512 + 1) * 512)
        res.append((a0, b1))
        a0 = b1
    return res


def _mask_arrays():
    """A [32,128] indicator; B patterns [32,128] (strict/incl); Bd8 [32,1024]."""
    A = np.zeros((32, P), np.float32)
    A[np.arange(P) // BS, np.arange(P)] = 1.0
    r = np.arange(32)[:, None]
    qb = (np.arange(P) // BS)[None, :]
    Bs = np.where(r >= qb, NEG, 0.0).astype(np.float32)   # xt q vs x0 k: allow r < qb
    Bi = np.where(r > qb, NEG, 0.0).astype(np.float32)    # x0 q vs x0 k: allow r <= qb
    Bd = np.where(r != qb, NEG, 0.0).astype(np.float32)   # xt q vs xt k: allow r == qb
    Bd8 = np.tile(Bd, (1, NT))                             # [32, 1024]
    bf = ml_dtypes.bfloat16
    return A.astype(bf), Bs.astype(bf), Bi.astype(bf), Bd8.astype(bf)


def _build():
    import concourse.tile_utils as tile_utils

    tile_utils.max_sbuf_usage = 204 * 1024

    nc = bacc.Bacc("TRN2", target_bir_lowering=False, debug=False, num_devices=8)

    # x slab-major [4, D, 512]; weights pair-major [PAIRS, D, 128]: every
    # input DMA reads a contiguous DRAM block
    xT = nc.dram_tensor("xT", [4, D, 512], BF16, kind="ExternalInput").ap()
    wq = nc.dram_tensor("wq", [PAIRS, D, P], BF16, kind="ExternalInput").ap()
    wk = nc.dram_tensor("wk", [PAIRS, D, P], BF16, kind="ExternalInput").ap()
    wv = nc.dram_tensor("wv", [PAIRS, D, P], BF16, kind="ExternalInput").ap()
    wo = nc.dram_tensor("wo", [D // 2, D], BF16, kind="ExternalInput").ap()
    bqs = nc.dram_tensor("bqs", [D // 2], F32, kind="ExternalInput").ap()
    out = nc.dram_tensor("out", [T, D], F32, kind="ExternalOutput").ap()
    scratch = nc.dram_tensor(
        "scratch", [16, L], F32, kind="ExternalOutput" if DBG else "Internal"
    ).ap()
    dbg = {}
    if DBG:
        for nm, shp, dt in (
            ("dbg_qT", [P, PAIRS, T], F32),
            ("dbg_kT", [P, PAIRS, T], F32),
            ("dbg_v", [P, PAIRS, T // P, 2 * (HD + 1)], F32),
            ("dbg_ctxT", [P, PAIRS, T], F32),
        ):
            dbg[nm] = nc.dram_tensor(nm, shp, dt, kind="ExternalOutput").ap()

    mA_np, mBs_np, mBi_np, mBd8_np = _mask_arrays()
    mA_d = nc.inline_tensor(mA_np, "mA_d").ap()
    mBs_d = nc.inline_tensor(mBs_np, "mBs_d").ap()
    mBi_d = nc.inline_tensor(mBi_np, "mBi_d").ap()
    mBd_d = nc.inline_tensor(mBd8_np, "mBd_d").ap()

    views = dict(
        xT_v=xT.rearrange("s (kc p) t -> p s kc t", p=P),  # [128, 4, 8, 512]
        wq_v=wq.rearrange("pr (kc p) m -> p pr kc m", p=P),  # [128, 4, 8, 128]
        wk_v=wk.rearrange("pr (kc p) m -> p pr kc m", p=P),
        wv_v=wv.rearrange("pr (kc p) m -> p pr kc m", p=P),
        wo_v=wo.rearrange("(cc p) n -> p cc n", p=P),      # [128, 4, 1024]
        bqs_v=bqs.rearrange("(c p) -> p c", p=P),          # [128, 4]
        mA=mA_d, mBs=mBs_d, mBi=mBi_d, mBd=mBd_d,
        out=out, scratch=scratch, dbg=dbg,
    )

    with tile.TileContext(nc) as tc:
        with tc.tile_pool(name="persist", bufs=1) as pers:
            st = dict(
                x_sb=pers.tile([P, KC, T], BF16, name="x_sb"),
                qT=pers.tile([P, PAIRS, T], BF16, name="qT"),
                kT=pers.tile([P, PAIRS, T], BF16, name="kT"),
                v_sb=pers.tile([P, PAIRS, T // P, 2 * (HD + 1)], BF16, name="v_sb"),
                ctxT=pers.tile([P, PAIRS, T], BF16, name="ctxT"),
                wq_sb=pers.tile([P, KC, D // 2], BF16, name="wq_sb"),
                wk_sb=pers.tile([P, KC, D // 2], BF16, name="wk_sb"),
                wv_sb=pers.tile([P, KC, D // 2], BF16, name="wv_sb"),
                wo_sb=pers.tile([P, PAIRS, D], BF16, name="wo_sb"),
                bq_sb=pers.tile([P, PAIRS], F32, name="bq_sb"),
                mA_sb=pers.tile([32, P], BF16, name="mA_sb"),
                mBs_sb=pers.tile([32, P], BF16, name="mBs_sb"),
                mBi_sb=pers.tile([32, P], BF16, name="mBi_sb"),
                mBd_sb=pers.tile([32, NT * P], BF16, name="mBd_sb"),
            )
            # ones columns (64 and 129) for the softmax denominators
            vv = st["v_sb"]
            nc.vector.memset(vv[:, :, :, HD : HD + 1], 1.0)
            nc.vector.memset(vv[:, :, :, 2 * HD + 1 : 2 * HD + 2], 1.0)
            ones_c = pers.tile([1, 1], F32, name="ones_c")
            nc.vector.memset(ones_c, 1.0)
            st["ones_t"] = pers.tile([1, HD], F32, name="ones_t")
            nc.vector.tensor_copy(
                st["ones_t"].bitcast(mybir.dt.float32r),
                ones_c[0:1, 0:1].to_broadcast((1, HD)),
            )
            nc.sync.dma_start(st["mA_sb"], views["mA"])
            nc.sync.dma_start(st["mBs_sb"], views["mBs"])
            nc.sync.dma_start(st["mBi_sb"], views["mBi"])
            nc.sync.dma_start(st["mBd_sb"], views["mBd"])
            nc.sync.dma_start(st["bq_sb"], views["bqs_v"])

            for _rep in range(REPEAT):
                _phases(nc, tc, st, views)

    nc.compile()
    return nc


def _phases(nc, tc, st, views):
    x_sb, qT, kT, v_sb, ctxT = (
        st["x_sb"], st["qT"], st["kT"], st["v_sb"], st["ctxT"]
    )
    scratch, out = views["scratch"], views["out"]

    # input DMAs, ordered by first use: pair-0 weights + x slab 0 first
    cols0 = slice(0, P)
    nc.sync.dma_start(st["wq_sb"][:, :, cols0], views["wq_v"][:, :, cols0])
    nc.sync.dma_start(st["wk_sb"][:, :, cols0], views["wk_v"][:, :, cols0])
    nc.sync.dma_start(st["wv_sb"][:, :, cols0], views["wv_v"][:, :, cols0])
    for s4 in range(4):  # slab 0 split fine so the first chunk lands early
        nc.sync.dma_start(
            x_sb[:, :, 128 * s4 : 128 * (s4 + 1)],
            views["xT_v"][:, :, 128 * s4 : 128 * (s4 + 1)],
        )
    for s in range(1, 4):
        nc.sync.dma_start(
            x_sb[:, :, 512 * s : 512 * (s + 1)],
            views["xT_v"][:, :, 512 * s : 512 * (s + 1)],
        )
    for p in range(1, PAIRS):
        cols = slice(P * p, P * (p + 1))
        nc.sync.dma_start(st["wq_sb"][:, :, cols], views["wq_v"][:, :, cols])
        nc.sync.dma_start(st["wk_sb"][:, :, cols], views["wk_v"][:, :, cols])
        nc.sync.dma_start(st["wv_sb"][:, :, cols], views["wv_v"][:, :, cols])
    nc.sync.dma_start(st["wo_sb"], views["wo_v"])

    from contextlib import ExitStack

    es = ExitStack()
    atp = es.enter_context(tc.tile_pool(name="atp", bufs=10))
    ctxs_p = es.enter_context(tc.tile_pool(name="ctxs_p", bufs=3))
    rbp = es.enter_context(tc.tile_pool(name="rbp", bufs=3))
    rcp = es.enter_context(tc.tile_pool(name="rcp", bufs=2))
    osbp = es.enter_context(tc.tile_pool(name="osbp", bufs=4))
    spsum = es.enter_context(tc.tile_pool(name="spsum", bufs=2, space="PSUM"))
    cpsum = es.enter_context(tc.tile_pool(name="cpsum", bufs=1, space="PSUM"))

    uid = [0]

    def nid():
        uid[0] += 1
        return uid[0]

    # ---------------- emission units ----------------

    def proj_units(apsum, p):
        """QKV projection for head-pair p as a list of small closures."""
        units = []
        cols = slice(P * p, P * (p + 1))

        def qk_chunk(dst, w_sb, tchunk, is_q):
            def go():
                ps = apsum.tile([P, 512], F32, tag="pp", name=f"pp{nid()}")
                for kc in range(KC):
                    nc.tensor.matmul(
                        ps,
                        w_sb[:, kc, cols],
                        x_sb[:, kc, 512 * tchunk : 512 * (tchunk + 1)],
                        start=(kc == 0),
                        stop=(kc == KC - 1),
                    )
                dv = dst[:, p, 512 * tchunk : 512 * (tchunk + 1)]
                if is_q:
                    nc.vector.tensor_scalar_add(dv, ps, st["bq_sb"][:, p : p + 1])
                else:
                    nc.vector.tensor_copy(dv, ps)
            return go

        def v_tile(tt):
            def go():
                ps = apsum.tile([P, P], F32, tag="pp", name=f"ppv{nid()}")
                xt = x_sb[:, :, P * tt : P * (tt + 1)]
                for kc in range(KC):
                    nc.tensor.matmul(
                        ps,
                        xt[:, kc, :],
                        st["wv_sb"][:, kc, cols],
                        start=(kc == 0),
                        stop=(kc == KC - 1),
                    )
                dv = v_sb[:, p, tt].rearrange("p (h c) -> p h c", c=HD + 1)[:, :, :HD]
                nc.vector.tensor_copy(dv, ps.rearrange("p (h c) -> p h c", c=HD))
            return go

        for tchunk in range(4):
            units.append(qk_chunk(qT, st["wq_sb"], tchunk, True))
            units.append(qk_chunk(kT, st["wk_sb"], tchunk, False))
            for t2 in range(4):
                units.append(v_tile(4 * tchunk + t2))
        return units

    def attn_head(h, ticks, fast_norm=False):
        """Emit attention for local head h; ticks[half]() at interleave slots."""
        p, r0 = h // 2, HD * (h % 2)
        qh = qT[r0 : r0 + HD, p, :]
        kh = kT[r0 : r0 + HD, p, :]
        vcol = (HD + 1) * (h % 2)

        for half in range(2):
            tick = ticks[half]
            ats = []
            # scores for all j (keys = x0 tiles)
            for j in range(NT):
                span = L - P * j
                kv = kh[:, L + P * j : L + P * (j + 1)]
                sc = spsum.tile([P, 1024], F32, tag="sc", name=f"sc{nid()}")
                q0 = L * half + P * j
                for c0 in range(0, span, 512):
                    c1 = min(span, c0 + 512)
                    nc.tensor.matmul(
                        sc[:, c0:c1], kv, qh[:, q0 + c0 : q0 + c1],
                        start=True, stop=True,
                    )
                mB = st["mBs_sb"] if half == 0 else st["mBi_sb"]
                nc.tensor.matmul(sc[:, 0:P], st["mA_sb"], mB, start=False, stop=True)
                at = atp.tile([P, 1024], BF16, tag="at", name=f"at{nid()}")[:, :span]
                nc.scalar.activation(at, sc[:, :span], Act.Exp)
                ats.append(at)
                if j % 2 == 1:
                    tick()
            # xt-xt block-diagonal scores (half 0 only)
            if half == 0:
                scd = spsum.tile([P, 1024], F32, tag="sc", name=f"scd{nid()}")
                for i in range(NT):
                    # start=True clears has_written for the WHOLE bank: only
                    # the first write per 512-col bank may set it
                    nc.tensor.matmul(
                        scd[:, P * i : P * (i + 1)],
                        kh[:, P * i : P * (i + 1)],
                        qh[:, P * i : P * (i + 1)],
                        start=(i % 4 == 0), stop=True,
                    )
                for c in range(2):
                    nc.tensor.matmul(
                        scd[:, 512 * c : 512 * (c + 1)],
                        st["mA_sb"],
                        st["mBd_sb"][:, 512 * c : 512 * (c + 1)],
                        start=False, stop=True,
                    )
                atd = atp.tile([P, 1024], BF16, tag="at", name=f"atd{nid()}")
                nc.scalar.activation(atd, scd, Act.Exp)
                tick()

            # ctx accumulation over j (+ diag for half 0); chunks at absolute
            # 512-boundaries of the ctx tile (PSUM banks)
            ctx = cpsum.tile([HD + 1, L], F32, tag="ctx", name=f"ctx{nid()}")
            for j in range(NT):
                vj = v_sb[:, p, NT + j, vcol : vcol + HD + 1]
                for a0, a1 in _chunks512(P * j, L):
                    c0 = a0 - P * j
                    last = half == 1 and (
                        (a1 <= 512 and j == 3) or (a0 >= 512 and j == NT - 1)
                    )
                    nc.tensor.matmul(
                        ctx[:, a0:a1], vj, ats[j][:, c0 : c0 + (a1 - a0)],
                        start=(j == 0), stop=last,
                    )
                if j % 2 == 1:
                    tick()
            if half == 0:
                for i in range(NT):
                    vi = v_sb[:, p, i, vcol : vcol + HD + 1]
                    nc.tensor.matmul(
                        ctx[:, P * i : P * (i + 1)], vi, atd[:, P * i : P * (i + 1)],
                        start=False, stop=(i == 3 or i == NT - 1),
                    )
                tick()

            # evict ctx, 1/denom, DRAM-trip partition broadcast, normalize.
            # Split the eviction per bank so ctx's PSUM frees earlier (WAR).
            ctx_s = ctxs_p.tile([HD + 1, L], F32, tag="cs", name=f"cs{nid()}")
            nc.vector.tensor_copy(ctx_s[:, 0:512], ctx[:, 0:512])
            nc.vector.tensor_copy(ctx_s[:, 512:L], ctx[:, 512:L])
            rc = rcp.tile([1, L], F32, tag="rc", name=f"rc{nid()}")
            dst = ctxT[r0 : r0 + HD, p, L * half : L * (half + 1)]
            F32R = mybir.dt.float32r
            if fast_norm:
                # tail fast path: PE broadcast of 1/denom (no DRAM trip)
                with nc.allow_low_precision(reason="deliberate f32r recip"):
                    nc.vector.reciprocal(rc.bitcast(F32R), ctx_s[HD : HD + 1, :])
                bc = spsum.tile([P, 1024], F32, tag="sc", name=f"bc{nid()}")
                for c0 in (0, 512):
                    nc.tensor.matmul(
                        bc[:HD, c0 : c0 + 512],
                        st["ones_t"].bitcast(F32R),
                        rc[0:1, c0 : c0 + 512].bitcast(F32R),
                        start=True, stop=True,
                    )
                nc.vector.tensor_mul(dst, ctx_s[:HD, :], bc[:HD, :])
            else:
                nc.vector.reciprocal(rc, ctx_s[HD : HD + 1, :])
                row = 2 * h + half
                nc.sync.dma_start(scratch[row : row + 1, :], rc)
                rb = rbp.tile([HD, L], F32, tag="rb", name=f"rb{nid()}")
                nc.sync.dma_start(rb, scratch[row : row + 1, :].to_broadcast((HD, L)))
                nc.vector.tensor_mul(dst, ctx_s[:HD, :], rb)
            tick()

    def oproj_units(opsum, half, dve_evict):
        """O-projection units for output rows of one half."""
        units = []

        def o_unit(tt, nk):
            def go():
                ops = opsum.tile([P, 512], F32, tag="op", name=f"op{nid()}")
                for cc in range(PAIRS):
                    nc.tensor.matmul(
                        ops,
                        ctxT[:, cc, P * tt : P * (tt + 1)],
                        st["wo_sb"][:, cc, 512 * nk : 512 * (nk + 1)],
                        start=(cc == 0),
                        stop=(cc == PAIRS - 1),
                    )
                osb = osbp.tile([P, 512], F32, tag="osb", name=f"osb{nid()}")
                if dve_evict:
                    nc.vector.tensor_copy(osb, ops)
                else:
                    nc.scalar.activation(osb, ops, Act.Copy)
                nc.sync.dma_start(
                    out[P * tt : P * (tt + 1), 512 * nk : 512 * (nk + 1)], osb
                )
            return go

        for tt in range(NT * half, NT * (half + 1)):
            for nk in range(2):
                units.append(o_unit(tt, nk))
        return units

    # ---------------- schedule ----------------

    def make_tick(pending):
        state = [0]

        def tick():
            state[0] += 1
            if pending and state[0] % 2 == 0:
                pending.pop(0)()
        return tick

    noop = lambda: None

    if INTERLEAVE:
        with tc.tile_pool(name="apsum", bufs=2, space="PSUM") as apsum:
            # warmup: keep the PE busy (and its clock hot) while the first
            # x-slab DMA lands; mask tiles arrive within ~1us
            for w in range(12):
                wps = apsum.tile([P, 512], F32, tag="pp", name=f"warm{nid()}")
                nc.tensor.matmul(
                    wps, st["mA_sb"], st["mBd_sb"][:, 0:512],
                    start=True, stop=True,
                )
            for u in proj_units(apsum, 0):
                u()
            for p in range(3):
                pending = proj_units(apsum, p + 1)
                tick = make_tick(pending)
                attn_head(2 * p, (tick, tick))
                attn_head(2 * p + 1, (tick, tick))
                for u in pending:
                    u()

        with tc.tile_pool(name="opsum", bufs=2, space="PSUM") as opsum:
            attn_head(6, (noop, noop))
            pending = oproj_units(opsum, 0, dve_evict=True)  # xt rows
            tick = make_tick(pending)
            attn_head(7, (noop, tick), fast_norm=True)
            for u in pending:
                u()
            for u in oproj_units(opsum, 1, dve_evict=False):  # x0 rows
                u()
    else:
        with tc.tile_pool(name="apsum", bufs=2, space="PSUM") as apsum:
            for p in range(4):
                for u in proj_units(apsum, p):
                    u()
        for h in range(8):
            attn_head(h, (noop, noop))
        with tc.tile_pool(name="opsum", bufs=2, space="PSUM") as opsum:
            for half in range(2):
                for u in oproj_units(opsum, half, dve_evict=False):
                    u()

    if views["dbg"]:
        dbg = views["dbg"]
        with tc.tile_pool(name="dbgp", bufs=2) as dbgp:
            for nm, src in (
                ("dbg_qT", qT), ("dbg_kT", kT), ("dbg_ctxT", ctxT)
            ):
                for pp in range(PAIRS):
                    t32 = dbgp.tile([P, T], F32, tag="d32", name=f"d{nid()}")
                    nc.vector.tensor_copy(t32, src[:, pp, :])
                    nc.sync.dma_start(dbg[nm][:, pp, :], t32)
            for pp in range(PAIRS):
                t32 = dbgp.tile([P, T // P * 2 * (HD + 1)], F32, tag="d32", name=f"dv{nid()}")
                tv = t32.rearrange("p (t c) -> p t c", c=2 * (HD + 1))
                nc.vector.tensor_copy(tv, v_sb[:, pp])
                nc.sync.dma_start(dbg["dbg_v"][:, pp], tv)

    es.close()


def kernel(x, Wq, bq, Wk, bk, Wv, bv, Wo, bo, block_size=4, **_):
    assert int(block_size) == BS
    bf = ml_dtypes.bfloat16
    x = np.asarray(x, np.float32)
    Wq, bq = np.asarray(Wq, np.float32), np.asarray(bq, np.float32)
    Wk = np.asarray(Wk, np.float32)
    Wv, bv = np.asarray(Wv, np.float32), np.asarray(bv, np.float32)
    Wo, bo = np.asarray(Wo, np.float32), np.asarray(bo, np.float32)

    if "nc" not in _CACHE:
        _CACHE["nc"] = _build()
    nc = _CACHE["nc"]

    scale = np.float32(HD ** -0.5)
    in_maps = []
    for core in range(8):
        b, g = core // 2, core % 2
        cols = slice(D // 2 * g, D // 2 * (g + 1))
        in_maps.append(
            {
                "xT": np.ascontiguousarray(x[b].T).astype(bf),
                "wq": np.ascontiguousarray(Wq[:, cols] * scale).astype(bf),
                "wk": np.ascontiguousarray(Wk[:, cols]).astype(bf),
                "wv": np.ascontiguousarray(Wv[:, cols]).astype(bf),
                "wo": np.ascontiguousarray(Wo[cols, :]).astype(bf),
                "bqs": np.ascontiguousarray(bq[cols]) * scale,
            }
        )

    _CACHE["last_in_maps"] = in_maps
    last_err = None
    for _attempt in range(6):
        try:
            res = run_bass_kernel_spmd(nc, in_maps, core_ids=list(range(8)), trace=False)
            break
        except Exception as e:  # transient NRT device flakes
            last_err = e
            msg = str(e)
            if "UNRECOVERABLE" not in msg and "UNAVAILABLE" not in msg:
                raise
            import time as _time

            import jax as _jax

            _time.sleep(5 * (_attempt + 1))
            try:
                _jax.clear_backends()
            except Exception:
                pass
    else:
        raise last_err

    corr = (bv @ Wo + bo).astype(np.float32)  # softmax rows sum to 1
    outv = np.empty((B, T, D), np.float32)
    for b in range(B):
        outv[b] = res.results[2 * b]["out"] + res.results[2 * b + 1]["out"] + corr
    return outv


if __name__ == "__main__":
    rng = np.random.default_rng(0)
    inputs = {
        "x": rng.standard_normal((B, T, D)).astype(np.float32),
        "Wq": (rng.standard_normal((D, D)) / 32).astype(np.float32),
        "bq": np.zeros(D, np.float32),
        "Wk": (rng.standard_normal((D, D)) / 32).astype(np.float32),
        "bk": np.zeros(D, np.float32),
        "Wv": (rng.standard_normal((D, D)) / 32).astype(np.float32),
        "bv": np.zeros(D, np.float32),
        "Wo": (rng.standard_normal((D, D)) / 32).astype(np.float32),
        "bo": np.zeros(D, np.float32),
    }
    o = kernel(**inputs)
    print("ran", o.shape, o.dtype, float(np.abs(o).max()))


# revision 29
# speedup vs baseline: 1.1477x; 1.0751x over previous
"""BD3LM block-diffusion decoder layer on 8 trn2 NeuronCores — v2 (bf16).

Sharding: core = 2*b + g  (b = batch 0..3, g = head-group 0..1, 8 heads each).
All matmuls bf16 (inputs converted on host; hd^-0.5 folded into Wq; bk dropped
— a per-query constant score shift is softmax-invariant). f32 PSUM accum.

Key structures vs v1:
  - Block masks applied ADDITIVELY on the PE: scores += A^T B where A[r,k] =
    [k//4 == r] (rank-32 indicator) and B[r,q] carries NEG on disallowed
    (block_r, q) pairs. One extra 128-col matmul per boundary tile; exp then
    covers a whole (head, half, j) span in ONE activation instruction.
  - Per-head-pair QKV projection, emission-interleaved with the previous
    pair's attention so the PE never waits on ACT's exp stream.
  - Softmax denominators via ones-columns in v (ctx row 64); reciprocal rows
    are broadcast across 64 partitions by a DRAM round-trip DMA; normalize is
    fused into the ctx eviction multiply on DVE.
  - Phase C (O-projection) interleaved into the last head's attention tail.
"""

import numpy as np
import ml_dtypes

import concourse.bass as bass
import concourse.mybir as mybir
import concourse.tile as tile
from concourse import bacc
from concourse.bass_utils import run_bass_kernel_spmd

F32 = mybir.dt.float32
BF16 = mybir.dt.bfloat16
Act = mybir.ActivationFunctionType

B, T, D = 4, 2048, 1024
H, HD = 16, 64
L = T // 2           # 1024
BS = 4               # block size
P = 128
NT = L // P          # 8 key/query tiles per half
PAIRS = 4            # head-pairs per core
KC = D // P          # 8 contraction chunks
NEG = -60.0          # additive mask value

REPEAT = 1  # loop whole computation inside the NEFF (timing experiments only)
DBG = False
INTERLEAVE = True

_CACHE = {}


def _chunks512(a0, a1):
    """Split [a0, a1) at multiples of 512 (PSUM bank boundaries)."""
    res = []
    while a0 < a1:
        b1 = min(a1, (a0 // 512 + 1) * 512)
        res.append((a0, b1))
        a0 = b1
    return res


def _mask_arrays():
    """A [32,128] indicator; B patterns [32,128] (strict/incl); Bd8 [32,1024]."""
    A = np.zeros((32, P), np.float32)
    A[np.arange(P) // BS, np.arange(P)] = 1.0
    r = np.arange(32)[:, None]
    qb = (np.arange(P) // BS)[None, :]
    Bs = np.where(r >= qb, NEG, 0.0).astype(np.float32)   # xt q vs x0 k: allow r < qb
    Bi = np.where(r > qb, NEG, 0.0).astype(np.float32)    # x0 q vs x0 k: allow r <= qb
    Bd = np.where(r != qb, NEG, 0.0).astype(np.float32)   # xt q vs xt k: allow r == qb
    Bd8 = np.tile(Bd, (1, NT))                             # [32, 1024]
    bf = ml_dtypes.bfloat16
    return A.astype(bf), Bs.astype(bf), Bi.astype(bf), Bd8.astype(bf)


def _build():
    import concourse.tile_utils as tile_utils

    tile_utils.max_sbuf_usage = 204 * 1024

    nc = bacc.Bacc("TRN2", target_bir_lowering=False, debug=False, num_devices=8)

    # x slab-major [4, D, 512]; weights pair-major [PAIRS, D, 128]: every
    # input DMA reads a contiguous DRAM block
    xT = nc.dram_tensor("xT", [4, D, 512], BF16, kind="ExternalInput").ap()
    wq = nc.dram_tensor("wq", [PAIRS, D, P], BF16, kind="ExternalInput").ap()
    wk = nc.dram_tensor("wk", [PAIRS, D, P], BF16, kind="ExternalInput").ap()
    wv = nc.dram_tensor("wv", [PAIRS, D, P], BF16, kind="ExternalInput").ap()
    wo = nc.dram_tensor("wo", [D // 2, D], BF16, kind="ExternalInput").ap()
    bqs = nc.dram_tensor("bqs", [D // 2], F32, kind="ExternalInput").ap()
    out = nc.dram_tensor("out", [T, D], F32, kind="ExternalOutput").ap()
    scratch = nc.dram_tensor(
        "scratch", [16, L], F32, kind="ExternalOutput" if DBG else "Internal"
    ).ap()
    dbg = {}
    if DBG:
        for nm, shp, dt in (
            ("dbg_qT", [P, PAIRS, T], F32),
            ("dbg_kT", [P, PAIRS, T], F32),
            ("dbg_v", [P, PAIRS, T // P, 2 * (HD + 1)], F32),
            ("dbg_ctxT", [P, PAIRS, T], F32),
        ):
            dbg[nm] = nc.dram_tensor(nm, shp, dt, kind="ExternalOutput").ap()

    mA_np, mBs_np, mBi_np, mBd8_np = _mask_arrays()
    mA_d = nc.inline_tensor(mA_np, "mA_d").ap()
    mBs_d = nc.inline_tensor(mBs_np, "mBs_d").ap()
    mBi_d = nc.inline_tensor(mBi_np, "mBi_d").ap()
    mBd_d = nc.inline_tensor(mBd8_np, "mBd_d").ap()

    views = dict(
        xT_v=xT.rearrange("s (kc p) t -> p s kc t", p=P),  # [128, 4, 8, 512]
        wq_v=wq.rearrange("pr (kc p) m -> p pr kc m", p=P),  # [128, 4, 8, 128]
        wk_v=wk.rearrange("pr (kc p) m -> p pr kc m", p=P),
        wv_v=wv.rearrange("pr (kc p) m -> p pr kc m", p=P),
        wo_v=wo.rearrange("(cc p) n -> p cc n", p=P),      # [128, 4, 1024]
        bqs_v=bqs.rearrange("(c p) -> p c", p=P),          # [128, 4]
        mA=mA_d, mBs=mBs_d, mBi=mBi_d, mBd=mBd_d,
        out=out, scratch=scratch, dbg=dbg,
    )

    with tile.TileContext(nc) as tc:
        with tc.tile_pool(name="persist", bufs=1) as pers:
            st = dict(
                x_sb=pers.tile([P, KC, T], BF16, name="x_sb"),
                qT=pers.tile([P, PAIRS, T], BF16, name="qT"),
                kT=pers.tile([P, PAIRS, T], BF16, name="kT"),
                v_sb=pers.tile([P, PAIRS, T // P, 2 * (HD + 1)], BF16, name="v_sb"),
                ctxT=pers.tile([P, PAIRS, T], BF16, name="ctxT"),
                wq_sb=pers.tile([P, KC, D // 2], BF16, name="wq_sb"),
                wk_sb=pers.tile([P, KC, D // 2], BF16, name="wk_sb"),
                wv_sb=pers.tile([P, KC, D // 2], BF16, name="wv_sb"),
                wo_sb=pers.tile([P, PAIRS, D], BF16, name="wo_sb"),
                bq_sb=pers.tile([P, PAIRS], F32, name="bq_sb"),
                mA_sb=pers.tile([32, P], BF16, name="mA_sb"),
                mBs_sb=pers.tile([32, P], BF16, name="mBs_sb"),
                mBi_sb=pers.tile([32, P], BF16, name="mBi_sb"),
                mBd_sb=pers.tile([32, NT * P], BF16, name="mBd_sb"),
            )
            # ones columns (64 and 129) for the softmax denominators
            vv = st["v_sb"]
            nc.vector.memset(vv[:, :, :, HD : HD + 1], 1.0)
            nc.vector.memset(vv[:, :, :, 2 * HD + 1 : 2 * HD + 2], 1.0)
            ones_c = pers.tile([1, 1], F32, name="ones_c")
            nc.vector.memset(ones_c, 1.0)
            st["ones_t"] = pers.tile([1, HD], F32, name="ones_t")
            nc.vector.tensor_copy(
                st["ones_t"].bitcast(mybir.dt.float32r),
                ones_c[0:1, 0:1].to_broadcast((1, HD)),
            )
            nc.sync.dma_start(st["mA_sb"], views["mA"])
            nc.sync.dma_start(st["mBs_sb"], views["mBs"])
            nc.sync.dma_start(st["mBi_sb"], views["mBi"])
            nc.sync.dma_start(st["mBd_sb"], views["mBd"])
            nc.sync.dma_start(st["bq_sb"], views["bqs_v"])

            from contextlib import ExitStack

            es = ExitStack()
            pools = dict(
                atp=es.enter_context(tc.tile_pool(name="atp", bufs=10)),
                ctxs_p=es.enter_context(tc.tile_pool(name="ctxs_p", bufs=3)),
                rbp=es.enter_context(tc.tile_pool(name="rbp", bufs=3)),
                rcp=es.enter_context(tc.tile_pool(name="rcp", bufs=2)),
                osbp=es.enter_context(tc.tile_pool(name="osbp", bufs=4)),
                spsum=es.enter_context(
                    tc.tile_pool(name="spsum", bufs=2, space="PSUM")
                ),
                cpsum=es.enter_context(
                    tc.tile_pool(name="cpsum", bufs=1, space="PSUM")
                ),
                gpsum=es.enter_context(
                    tc.tile_pool(name="gpsum", bufs=2, space="PSUM")
                ),
            )
            for _rep in range(REPEAT):
                _phases(nc, tc, st, views, pools, warm=(_rep == 0))
            es.close()

    nc.compile()
    return nc


def _phases(nc, tc, st, views, pools, warm=False):
    x_sb, qT, kT, v_sb, ctxT = (
        st["x_sb"], st["qT"], st["kT"], st["v_sb"], st["ctxT"]
    )
    scratch, out = views["scratch"], views["out"]
    atp, ctxs_p, rbp, rcp, osbp = (
        pools["atp"], pools["ctxs_p"], pools["rbp"], pools["rcp"], pools["osbp"]
    )
    spsum, cpsum, gpsum = pools["spsum"], pools["cpsum"], pools["gpsum"]

    # input DMAs, ordered by first use: pair-0 weights + x slab 0 first.
    # All sources are contiguous DRAM blocks (slab-major x, pair-major w).
    nc.sync.dma_start(st["wq_sb"][:, :, 0:P], views["wq_v"][:, 0])
    nc.sync.dma_start(st["wk_sb"][:, :, 0:P], views["wk_v"][:, 0])
    nc.sync.dma_start(st["wv_sb"][:, :, 0:P], views["wv_v"][:, 0])
    # slab 0 in two contiguous row-halves (kc 0-3, 4-7) so the first
    # projection chunks can start sooner
    nc.sync.dma_start(x_sb[:, 0:4, 0:512], views["xT_v"][:, 0, 0:4, :])
    nc.sync.dma_start(x_sb[:, 4:8, 0:512], views["xT_v"][:, 0, 4:8, :])
    for s in range(1, 4):
        nc.sync.dma_start(
            x_sb[:, :, 512 * s : 512 * (s + 1)], views["xT_v"][:, s]
        )
    for p in range(1, PAIRS):
        cols = slice(P * p, P * (p + 1))
        nc.sync.dma_start(st["wq_sb"][:, :, cols], views["wq_v"][:, p])
        nc.sync.dma_start(st["wk_sb"][:, :, cols], views["wk_v"][:, p])
        nc.sync.dma_start(st["wv_sb"][:, :, cols], views["wv_v"][:, p])
    nc.sync.dma_start(st["wo_sb"], views["wo_v"])

    uid = [0]

    def nid():
        uid[0] += 1
        return uid[0]

    # ---------------- emission units ----------------

    def proj_units(apsum, p):
        """QKV projection for head-pair p as a list of small closures."""
        units = []
        cols = slice(P * p, P * (p + 1))

        def qk_chunk(dst, w_sb, tchunk, is_q):
            def go():
                ps = apsum.tile([P, 512], F32, tag="pp", name=f"pp{nid()}")
                for kc in range(KC):
                    nc.tensor.matmul(
                        ps,
                        w_sb[:, kc, cols],
                        x_sb[:, kc, 512 * tchunk : 512 * (tchunk + 1)],
                        start=(kc == 0),
                        stop=(kc == KC - 1),
                    )
                dv = dst[:, p, 512 * tchunk : 512 * (tchunk + 1)]
                if is_q:
                    nc.vector.tensor_scalar_add(dv, ps, st["bq_sb"][:, p : p + 1])
                else:
                    nc.vector.tensor_copy(dv, ps)
            return go

        def v_tile(tt):
            def go():
                ps = apsum.tile([P, P], F32, tag="pp", name=f"ppv{nid()}")
                xt = x_sb[:, :, P * tt : P * (tt + 1)]
                for kc in range(KC):
                    nc.tensor.matmul(
                        ps,
                        xt[:, kc, :],
                        st["wv_sb"][:, kc, cols],
                        start=(kc == 0),
                        stop=(kc == KC - 1),
                    )
                dv = v_sb[:, p, tt].rearrange("p (h c) -> p h c", c=HD + 1)[:, :, :HD]
                nc.vector.tensor_copy(dv, ps.rearrange("p (h c) -> p h c", c=HD))
            return go

        for tchunk in range(4):
            units.append(qk_chunk(qT, st["wq_sb"], tchunk, True))
            units.append(qk_chunk(kT, st["wk_sb"], tchunk, False))
            for t2 in range(4):
                units.append(v_tile(4 * tchunk + t2))
        return units

    def attn_head(h, ticks, fast_norm=False):
        """Emit attention for local head h; ticks[half]() at interleave slots."""
        p, r0 = h // 2, HD * (h % 2)
        qh = qT[r0 : r0 + HD, p, :]
        kh = kT[r0 : r0 + HD, p, :]
        vcol = (HD + 1) * (h % 2)

        for half in range(2):
            tick = ticks[half]
            ats = []
            # scores for all j (keys = x0 tiles)
            for j in range(NT):
                span = L - P * j
                kv = kh[:, L + P * j : L + P * (j + 1)]
                sc = spsum.tile([P, 1024], F32, tag="sc", name=f"sc{nid()}")
                q0 = L * half + P * j
                for c0 in range(0, span, 512):
                    c1 = min(span, c0 + 512)
                    nc.tensor.matmul(
                        sc[:, c0:c1], kv, qh[:, q0 + c0 : q0 + c1],
                        start=True, stop=True,
                    )
                mB = st["mBs_sb"] if half == 0 else st["mBi_sb"]
                nc.tensor.matmul(sc[:, 0:P], st["mA_sb"], mB, start=False, stop=True)
                at = atp.tile([P, 1024], BF16, tag="at", name=f"at{nid()}")[:, :span]
                nc.scalar.activation(at, sc[:, :span], Act.Exp)
                ats.append(at)
                if j % 2 == 1:
                    tick()
            # xt-xt block-diagonal scores (half 0 only)
            if half == 0:
                scd = spsum.tile([P, 1024], F32, tag="sc", name=f"scd{nid()}")
                for i in range(NT):
                    # start=True clears has_written for the WHOLE bank: only
                    # the first write per 512-col bank may set it
                    nc.tensor.matmul(
                        scd[:, P * i : P * (i + 1)],
                        kh[:, P * i : P * (i + 1)],
                        qh[:, P * i : P * (i + 1)],
                        start=(i % 4 == 0), stop=True,
                    )
                for c in range(2):
                    nc.tensor.matmul(
                        scd[:, 512 * c : 512 * (c + 1)],
                        st["mA_sb"],
                        st["mBd_sb"][:, 512 * c : 512 * (c + 1)],
                        start=False, stop=True,
                    )
                atd = atp.tile([P, 1024], BF16, tag="at", name=f"atd{nid()}")
                nc.scalar.activation(atd, scd, Act.Exp)
                tick()

            # ctx accumulation over j (+ diag for half 0); chunks at absolute
            # 512-boundaries of the ctx tile (PSUM banks)
            ctx = cpsum.tile([HD + 1, L], F32, tag="ctx", name=f"ctx{nid()}")
            for j in range(NT):
                vj = v_sb[:, p, NT + j, vcol : vcol + HD + 1]
                for a0, a1 in _chunks512(P * j, L):
                    c0 = a0 - P * j
                    last = half == 1 and (
                        (a1 <= 512 and j == 3) or (a0 >= 512 and j == NT - 1)
                    )
                    nc.tensor.matmul(
                        ctx[:, a0:a1], vj, ats[j][:, c0 : c0 + (a1 - a0)],
                        start=(j == 0), stop=last,
                    )
                if j % 2 == 1:
                    tick()
            if half == 0:
                for i in range(NT):
                    vi = v_sb[:, p, i, vcol : vcol + HD + 1]
                    nc.tensor.matmul(
                        ctx[:, P * i : P * (i + 1)], vi, atd[:, P * i : P * (i + 1)],
                        start=False, stop=(i == 3 or i == NT - 1),
                    )
                tick()

            # evict ctx, 1/denom, DRAM-trip partition broadcast, normalize.
            # Split the eviction per bank so ctx's PSUM frees earlier (WAR).
            ctx_s = ctxs_p.tile([HD + 1, L], F32, tag="cs", name=f"cs{nid()}")
            nc.vector.tensor_copy(ctx_s[:, 0:512], ctx[:, 0:512])
            nc.vector.tensor_copy(ctx_s[:, 512:L], ctx[:, 512:L])
            rc = rcp.tile([1, L], F32, tag="rc", name=f"rc{nid()}")
            dst = ctxT[r0 : r0 + HD, p, L * half : L * (half + 1)]
            F32R = mybir.dt.float32r
            if fast_norm:
                # tail fast path: PE broadcast of 1/denom (no DRAM trip)
                with nc.allow_low_precision(reason="deliberate f32r recip"):
                    nc.vector.reciprocal(rc.bitcast(F32R), ctx_s[HD : HD + 1, :])
                bc = spsum.tile([P, 1024], F32, tag="sc", name=f"bc{nid()}")
                for c0 in (0, 512):
                    nc.tensor.matmul(
                        bc[:HD, c0 : c0 + 512],
                        st["ones_t"].bitcast(F32R),
                        rc[0:1, c0 : c0 + 512].bitcast(F32R),
                        start=True, stop=True,
                    )
                nc.vector.tensor_mul(dst, ctx_s[:HD, :], bc[:HD, :])
            else:
                nc.vector.reciprocal(rc, ctx_s[HD : HD + 1, :])
                row = 2 * h + half
                nc.sync.dma_start(scratch[row : row + 1, :], rc)
                rb = rbp.tile([HD, L], F32, tag="rb", name=f"rb{nid()}")
                nc.sync.dma_start(rb, scratch[row : row + 1, :].to_broadcast((HD, L)))
                nc.vector.tensor_mul(dst, ctx_s[:HD, :], rb)
            tick()

    def oproj_units(opsum, half, dve_evict):
        """O-projection units for output rows of one half."""
        units = []

        def o_unit(tt, nk):
            def go():
                ops = opsum.tile([P, 512], F32, tag="pp", name=f"op{nid()}")
                for cc in range(PAIRS):
                    nc.tensor.matmul(
                        ops,
                        ctxT[:, cc, P * tt : P * (tt + 1)],
                        st["wo_sb"][:, cc, 512 * nk : 512 * (nk + 1)],
                        start=(cc == 0),
                        stop=(cc == PAIRS - 1),
                    )
                osb = osbp.tile([P, 512], F32, tag="osb", name=f"osb{nid()}")
                if dve_evict:
                    nc.vector.tensor_copy(osb, ops)
                else:
                    nc.scalar.activation(osb, ops, Act.Copy)
                nc.sync.dma_start(
                    out[P * tt : P * (tt + 1), 512 * nk : 512 * (nk + 1)], osb
                )
            return go

        for tt in range(NT * half, NT * (half + 1)):
            for nk in range(2):
                units.append(o_unit(tt, nk))
        return units

    # ---------------- schedule ----------------

    def make_tick(pending):
        state = [0]

        def tick():
            state[0] += 1
            if pending and state[0] % 2 == 0:
                pending.pop(0)()
        return tick

    noop = lambda: None

    if INTERLEAVE:
        if warm:
            # warmup: keep the PE busy (and its clock hot) while the first
            # x-slab DMA lands; mask tiles arrive within ~1us
            for w in range(12):
                wps = gpsum.tile([P, 512], F32, tag="pp", name=f"warm{nid()}")
                nc.tensor.matmul(
                    wps, st["mA_sb"], st["mBd_sb"][:, 0:512],
                    start=True, stop=True,
                )
        for u in proj_units(gpsum, 0):
            u()
        for p in range(3):
            pending = proj_units(gpsum, p + 1)
            tick = make_tick(pending)
            attn_head(2 * p, (tick, tick))
            attn_head(2 * p + 1, (tick, tick))
            for u in pending:
                u()

        attn_head(6, (noop, noop))
        pending = oproj_units(gpsum, 0, dve_evict=True)  # xt rows
        tick = make_tick(pending)
        attn_head(7, (noop, tick), fast_norm=True)
        for u in pending:
            u()
        for u in oproj_units(gpsum, 1, dve_evict=False):  # x0 rows
            u()
    else:
        for p in range(4):
            for u in proj_units(gpsum, p):
                u()
        for h in range(8):
            attn_head(h, (noop, noop))
        for half in range(2):
            for u in oproj_units(gpsum, half, dve_evict=False):
                u()

    if views["dbg"]:
        dbg = views["dbg"]
        with tc.tile_pool(name="dbgp", bufs=2) as dbgp:
            for nm, src in (
                ("dbg_qT", qT), ("dbg_kT", kT), ("dbg_ctxT", ctxT)
            ):
                for pp in range(PAIRS):
                    t32 = dbgp.tile([P, T], F32, tag="d32", name=f"d{nid()}")
                    nc.vector.tensor_copy(t32, src[:, pp, :])
                    nc.sync.dma_start(dbg[nm][:, pp, :], t32)
            for pp in range(PAIRS):
                t32 = dbgp.tile([P, T // P * 2 * (HD + 1)], F32, tag="d32", name=f"dv{nid()}")
                tv = t32.rearrange("p (t c) -> p t c", c=2 * (HD + 1))
                nc.vector.tensor_copy(tv, v_sb[:, pp])
                nc.sync.dma_start(dbg["dbg_v"][:, pp], tv)


def kernel(x, Wq, bq, Wk, bk, Wv, bv, Wo, bo, block_size=4, **_):
    assert int(block_size) == BS
    bf = ml_dtypes.bfloat16
    x = np.asarray(x, np.float32)
    Wq, bq = np.asarray(Wq, np.float32), np.asarray(bq, np.float32)
    Wk = np.asarray(Wk, np.float32)
    Wv, bv = np.asarray(Wv, np.float32), np.asarray(bv, np.float32)
    Wo, bo = np.asarray(Wo, np.float32), np.asarray(bo, np.float32)

    if "nc" not in _CACHE:
        _CACHE["nc"] = _build()
    nc = _CACHE["nc"]

    scale = np.float32(HD ** -0.5)

    def slabbed(xt):  # [D, T] -> [4, D, 512] slab-major
        return np.ascontiguousarray(
            xt.reshape(D, 4, 512).transpose(1, 0, 2)
        )

    def pair_major(w):  # [D, 512] -> [4, D, 128]
        return np.ascontiguousarray(w.reshape(D, PAIRS, P).transpose(1, 0, 2))

    in_maps = []
    for core in range(8):
        b, g = core // 2, core % 2
        cols = slice(D // 2 * g, D // 2 * (g + 1))
        in_maps.append(
            {
                "xT": slabbed(x[b].T).astype(bf),
                "wq": pair_major(Wq[:, cols] * scale).astype(bf),
                "wk": pair_major(Wk[:, cols]).astype(bf),
                "wv": pair_major(Wv[:, cols]).astype(bf),
                "wo": np.ascontiguousarray(Wo[cols, :]).astype(bf),
                "bqs": np.ascontiguousarray(bq[cols]) * scale,
            }
        )

    _CACHE["last_in_maps"] = in_maps
    last_err = None
    for _attempt in range(6):
        try:
            res = run_bass_kernel_spmd(nc, in_maps, core_ids=list(range(8)), trace=False)
            break
        except Exception as e:  # transient NRT device flakes
            last_err = e
            msg = str(e)
            if "UNRECOVERABLE" not in msg and "UNAVAILABLE" not in msg:
                raise
            import time as _time

            import jax as _jax

            _time.sleep(5 * (_attempt + 1))
            try:
                _jax.clear_backends()
            except Exception:
                pass
    else:
        raise last_err

    corr = (bv @ Wo + bo).astype(np.float32)  # softmax rows sum to 1
    outv = np.empty((B, T, D), np.float32)
    for b in range(B):
        outv[b] = res.results[2 * b]["out"] + res.results[2 * b + 1]["out"] + corr
    return outv


if __name__ == "__main__":
    rng = np.random.default_rng(0)
    inputs = {
        "x": rng.standard_normal((B, T, D)).astype(np.float32),
        "Wq": (rng.standard_normal((D, D)) / 32).astype(np.float32),
        "bq": np.zeros(D, np.float32),
        "Wk": (rng.standard_normal((D, D)) / 32).astype(np.float32),
        "bk": np.zeros(D, np.float32),
        "Wv": (rng.standard_normal((D, D)) / 32).astype(np.float32),
        "bv": np.zeros(D, np.float32),
        "Wo": (rng.standard_normal((D, D)) / 32).astype(np.float32),
        "bo": np.zeros(D, np.float32),
    }
    o = kernel(**inputs)
    print("ran", o.shape, o.dtype, float(np.abs(o).max()))
